# revision 3
# baseline (speedup 1.0000x reference)
"""Causal self-attention on 8 Trainium2 NeuronCores.

Sharding (data + head parallel): core c handles batch b = c // 4 and the
4 heads [4g, 4g+4) where g = c % 4.  Each core projects q/k/v for its
heads (weights pre-sliced + pre-transposed on host), runs causal
attention, then the 4 cores of each batch AllGather the per-head
attention outputs (hd-major fp16) and each computes a disjoint
256-channel column slice of the output projection.

Pipeline (v2):
- x is DMA'd in 512-column slabs; projections run per-slab so the first
  attention chunk starts ~5us in instead of waiting for the full 4MB.
- Attention chunks interleave pairs hb-major so the four AllGathers fire
  spread across the kernel instead of piling up at the end; the output
  projection is split per-pair (SBUF f32 accumulator) so each half runs
  as soon as its gather lands, hidden under remaining attention.
- Softmax normalize reads PSUM directly: one reciprocal_approx_fast on
  the [1, 1024] denominator row, one gpsimd partition-broadcast, one
  fused multiply+cast-to-f16, then 2 DMAs to the collective buffer.
- fp16 data path, fp32 PSUM accumulation, f16 output (host casts).
- PSUM budget (8 banks): score tile 2 banks x 2 bufs, attention
  accumulators alternate the "at"/"pp" 2-bank slots with the projection
  matmuls.

Layouts per core:
  xT    (1024, 2048)  x[b].T                       (d on partitions)
  wqkT  (1024, 512)   [ (Wq[rows]/8).T | Wk[rows].T ]
  wvT   (1024, 256)   Wv[rows].T
  woT   (1024, 256)   Wo[rows].T with rows permuted to the AllGather
                      order: [pair p=0: rank r: heads 4r,4r+1] then
                      [pair p=1: rank r: heads 4r+2,4r+3]
  mask  (128, 128)    upper-triangular ones (k <= q)
  outT  (256, 2048)   out[b][:, cols].T  (f16)
"""

import numpy as np

B, S, D, H = 2, 2048, 1024, 16
HD = D // H              # 64
NCORES = 8
GROUP = 4                # cores per batch
LHEADS = 4               # heads per core
LCH = LHEADS * HD        # 256 local channels
KT = D // 128            # 8 contraction tiles
ST = S // 128            # 16 sequence tiles
PAIRS = 2                # head pairs per core
CHUNK = 512              # q columns per attention pass
NCH = S // CHUNK         # 4
GATH = 1024              # columns per collective
NHB = S // GATH          # 2 gather blocks

_CACHE = {}


def _f16(a):
    return np.ascontiguousarray(a, dtype=np.float16)


def _build():
    import concourse.bacc as bacc
    import concourse.mybir as mybir
    import concourse.tile as tile

    f32 = mybir.dt.float32
    f16 = mybir.dt.float16
    Exp = mybir.ActivationFunctionType.Exp

    nc = bacc.Bacc(num_devices=NCORES)
    xT = nc.dram_tensor("xT", [D, S], f16, kind="ExternalInput")
    wqkT = nc.dram_tensor("wqkT", [D, 2 * LCH], f16, kind="ExternalInput")
    wvT = nc.dram_tensor("wvT", [D, LCH], f16, kind="ExternalInput")
    woT = nc.dram_tensor("woT", [D, LCH], f16, kind="ExternalInput")
    mask = nc.dram_tensor("mask", [128, 128], f16, kind="ExternalInput")
    outT = nc.dram_tensor("outT", [LCH, S], f16, kind="ExternalOutput")

    RG = [[0, 1, 2, 3], [4, 5, 6, 7]]

    with tile.TileContext(nc, num_cores=NCORES) as tc:
        with (
            tc.tile_pool(name="const", bufs=1) as const,
            tc.tile_pool(name="qkv", bufs=1) as qkv,
            tc.tile_pool(name="psum", bufs=1, space="PSUM") as psum,
            tc.tile_pool(name="dram", bufs=1, space="DRAM") as dram,
            tc.tile_pool(name="work", bufs=1) as work,
            tc.tile_pool(name="proj", bufs=1) as projp,
            tc.tile_pool(name="agp", bufs=1) as agp,
        ):
            mask_sb = const.tile([128, 128], f16)
            nc.sync.dma_start(mask_sb[:], mask[:])
            ones4 = const.tile([128, LHEADS], f32)
            nc.vector.memset(ones4[:], 1.0)

            cc_in = [[dram.tile([128, GATH], f16, name=f"ccin{p}{hb}")
                      for hb in range(NHB)] for p in range(PAIRS)]
            cc_out = [[dram.tile([GROUP * 128, GATH], f16, name=f"ccout{p}{hb}")
                       for hb in range(NHB)] for p in range(PAIRS)]

            qt = qkv.tile([128, PAIRS, S], f16)
            kt = qkv.tile([128, PAIRS, S], f16)
            v = qkv.tile([128, ST, LHEADS, 65], f16)

            # ---------------- input loads (slab-major for x) ----------------
            wqk, wv, xt = [], [], []
            for k in range(KT):
                tw = projp.tile([128, 2 * LCH], f16, name=f"wqk{k}")
                nc.sync.dma_start(tw[:], wqkT[128 * k:128 * k + 128, :])
                wqk.append(tw)
            for k in range(KT):
                tx = projp.tile([128, NCH, CHUNK], f16, name=f"xt{k}")
                xt.append(tx)
            for s in range(NCH):
                for k in range(KT):
                    nc.sync.dma_start(
                        xt[k][:, s, :],
                        xT[128 * k:128 * k + 128, CHUNK * s:CHUNK * s + CHUNK])
                if s == 0:
                    for k in range(KT):
                        tv = projp.tile([128, LCH], f16, name=f"wv{k}")
                        nc.sync.dma_start(tv[:], wvT[128 * k:128 * k + 128, :])
                        wv.append(tv)
            wo = projp.tile([128, KT, LCH], f16)
            nc.sync.dma_start(wo[:], woT[:].rearrange("(k p) n -> p k n", p=128))

            ptag = [0]

            def next_tag():
                t = "at" if ptag[0] % 2 == 0 else "pp"
                ptag[0] += 1
                return t

            def qk_proj(m, s):
                # m: 0,1 = q pair 0/1; 2,3 = k pair 0/1; s: 512-col slab
                dst = qt if m < 2 else kt
                pp = psum.tile([128, CHUNK], f32, tag=next_tag(),
                               name=f"qk{m}{s}")
                for k in range(KT):
                    nc.tensor.matmul(
                        pp[:], wqk[k][:, 128 * m:128 * m + 128], xt[k][:, s, :],
                        start=(k == 0), stop=(k == KT - 1))
                nc.vector.tensor_copy(
                    dst[:, m % 2, CHUNK * s:CHUNK * s + CHUNK], pp[:])

            def v_proj(j):
                vps = psum.tile([128, LCH], f32, tag=next_tag(), name=f"v{j}")
                for k in range(KT):
                    nc.tensor.matmul(
                        vps[:], xt[k][:, j // 4, 128 * (j % 4):128 * (j % 4) + 128],
                        wv[k][:], start=(k == 0), stop=(k == KT - 1))
                nc.vector.tensor_copy(
                    v[:, j, :, 64:65], ones4[:].rearrange("p (h o) -> p h o", o=1))
                nc.vector.tensor_copy(
                    v[:, j, :, 0:64], vps[:].rearrange("p (h e) -> p h e", h=LHEADS))

            ccin_last = [None]

            def stage_chunk(p, c, attps):
                """Normalize chunk c's accumulators and ship to the collective
                buffer; AllGather once a 1024-col block is done."""
                hb, sub = c // 2, c % 2
                for h in range(2):
                    asb = work.tile([65, CHUNK], f32, tag=f"asb{h}", bufs=2,
                                    name=f"asb{p}{c}{h}")
                    nc.vector.tensor_copy(asb[:], attps[:, 512 * h:512 * h + CHUNK])
                    rc = work.tile([65, CHUNK], f32, tag="rc", bufs=2,
                                   name=f"rc{p}{c}{h}")
                    nc.vector.reciprocal(rc[64:65, :], asb[64:65, :])
                    rc0 = work.tile([1, CHUNK], f32, tag="rc0", bufs=2,
                                    name=f"rc0{p}{c}{h}")
                    nc.sync.dma_start(rc0[0:1, :], rc[64:65, :])
                    bc = work.tile([64, CHUNK], f32, tag="bc", bufs=2,
                                   name=f"bc{p}{c}{h}")
                    nc.gpsimd.partition_broadcast(bc[:], rc0[0:1, :])
                    ao = work.tile([64, CHUNK], f16, tag="ao", bufs=2,
                                   name=f"ao{p}{c}{h}")
                    nc.vector.tensor_mul(ao[:, :], asb[0:64, :], bc[:, :])
                    ccin_last[0] = nc.sync.dma_start(
                        cc_in[p][hb][64 * h:64 * h + 64,
                                     CHUNK * sub:CHUNK * sub + CHUNK], ao[:, :])
                if sub == 1:
                    nc.gpsimd.collective_compute(
                        "AllGather", mybir.AluOpType.bypass, replica_groups=RG,
                        ins=[cc_in[p][hb][:]], outs=[cc_out[p][hb][:]])

            def attn_chunk(p, c):
                q0 = CHUNK * c
                nj = 4 * c + 4
                attps = psum.tile([65, 2 * CHUNK], f32, tag=next_tag(),
                                  name=f"att{p}{c}")
                for j in range(nj):
                    qs = max(q0, 128 * j)
                    n = q0 + CHUNK - qs
                    off = qs - q0
                    sc = psum.tile([128, 1024], f32, tag="sc", bufs=2,
                                   name=f"sc{p}{c}{j}")
                    for h in range(2):
                        pb = 64 * h
                        nc.tensor.matmul(
                            sc[:, 512 * h:512 * h + n],
                            kt[pb:pb + 64, p, 128 * j:128 * j + 128],
                            qt[pb:pb + 64, p, qs:qs + n],
                            start=True, stop=True)
                    ex = work.tile([128, 1024], f16, tag="ex", bufs=3,
                                   name=f"ex{p}{c}{j}")
                    nc.scalar.activation(
                        ex[:].rearrange("q (t x) -> q t x", t=2)[:, :, 0:n],
                        sc[:].rearrange("q (t x) -> q t x", t=2)[:, :, 0:n],
                        Exp)
                    if qs == 128 * j:  # diagonal tile: causal mask
                        for h in range(2):
                            nc.vector.tensor_mul(
                                ex[:, 512 * h:512 * h + 128],
                                ex[:, 512 * h:512 * h + 128], mask_sb[:])
                    for h in range(2):
                        nc.tensor.matmul(
                            attps[:, 512 * h + off:512 * h + CHUNK],
                            v[:, j, 2 * p + h, :],
                            ex[:, 512 * h:512 * h + n],
                            start=(j == 0), stop=(j == nj - 1))
                stage_chunk(p, c, attps)

            oacc = [agp.tile([128, GATH], f32, name=f"oacc{ct}")
                    for ct in range(2)]

            def out_half(hb, ph, first):
                """Half of the output projection for gather block hb using
                pair ph's gathered heads.  first -> park in SBUF f32;
                second -> add, cast to f16 and write out."""
                ag = []
                for r in range(GROUP):
                    t = agp.tile([128, GATH], f16, tag=f"ag{ph}{r}", bufs=1,
                                 name=f"ag{ph}{hb}{r}")
                    dma = nc.sync.dma_start(
                        t[:], cc_out[ph][hb][128 * r:128 * r + 128, :])
                    if ccin_last[0] is not None:
                        tile.add_dep_helper(
                            dma.ins, ccin_last[0].ins, sync=True,
                            reason="gather prefetch after prior staging")
                    ag.append(t)
                for ct in range(2):
                    pp = psum.tile([128, GATH], f32, tag=next_tag(),
                                   name=f"op{hb}{ph}{ct}")
                    for k4 in range(GROUP):
                        for c2 in range(2):
                            nc.tensor.matmul(
                                pp[:, 512 * c2:512 * c2 + 512],
                                wo[:, 4 * ph + k4, 128 * ct:128 * ct + 128],
                                ag[k4][:, 512 * c2:512 * c2 + 512],
                                start=(k4 == 0), stop=(k4 == GROUP - 1))
                    if first:
                        nc.vector.tensor_copy(oacc[ct][:], pp[:])
                    else:
                        ot = agp.tile([128, GATH], f16, tag=f"ot{ct}", bufs=2,
                                      name=f"ot{hb}{ct}")
                        nc.vector.tensor_add(ot[:], pp[:], oacc[ct][:])
                        nc.sync.dma_start(
                            outT[128 * ct:128 * ct + 128,
                                 GATH * hb:GATH * hb + GATH], ot[:])

            # ---------------- schedule ----------------
            def proj_slab(s):
                qk_proj(0, s)         # pair-0 q
                qk_proj(2, s)         # pair-0 k
                for j in range(4 * s, 4 * s + 4):
                    v_proj(j)
                qk_proj(1, s)         # pair-1 q
                qk_proj(3, s)         # pair-1 k

            proj_slab(0)
            attn_chunk(0, 0)
            proj_slab(1)
            attn_chunk(1, 0)
            proj_slab(2)
            attn_chunk(0, 1)          # -> AG(p0, hb0)
            proj_slab(3)
            attn_chunk(1, 1)          # -> AG(p1, hb0)
            attn_chunk(1, 2)
            out_half(0, 0, True)      # after AG(p0, hb0)
            attn_chunk(1, 3)          # -> AG(p1, hb1)
            out_half(0, 1, False)     # after AG(p1, hb0); writes hb0
            attn_chunk(0, 2)
            attn_chunk(0, 3)          # -> AG(p0, hb1)
            out_half(1, 1, True)      # after AG(p1, hb1)
            out_half(1, 0, False)     # after AG(p0, hb1); writes hb1

    nc.compile()
    return nc


def _gather_perm():
    """d-channel permutation matching the AllGather layout."""
    perm = []
    for p in range(PAIRS):
        for r in range(GROUP):
            for h in range(2):
                head = 4 * r + 2 * p + h
                perm.extend(range(HD * head, HD * head + HD))
    return np.array(perm)


def _shard_inputs(x, Wq, Wk, Wv, Wo):
    x = np.asarray(x, dtype=np.float32)
    Wq = np.asarray(Wq, dtype=np.float32)
    Wk = np.asarray(Wk, dtype=np.float32)
    Wv = np.asarray(Wv, dtype=np.float32)
    Wo = np.asarray(Wo, dtype=np.float32)
    mask = np.triu(np.ones((128, 128), dtype=np.float16))
    perm = _gather_perm()
    in_maps = []
    for c in range(NCORES):
        b, g = c // GROUP, c % GROUP
        rows = slice(LCH * g, LCH * g + LCH)
        in_maps.append({
            "xT": _f16(x[b].T),
            "wqkT": _f16(np.concatenate([Wq[rows] / 8.0, Wk[rows]], axis=0).T),
            "wvT": _f16(Wv[rows].T),
            "woT": _f16(Wo[rows].T[perm, :]),
            "mask": mask,
        })
    return in_maps


def kernel(x, Wq, Wk, Wv, Wo):
    from concourse.bass_utils import run_bass_kernel_spmd

    if "nc" not in _CACHE:
        _CACHE["nc"] = _build()
    nc = _CACHE["nc"]
    in_maps = _shard_inputs(x, Wq, Wk, Wv, Wo)
    res = run_bass_kernel_spmd(nc, in_maps, core_ids=list(range(NCORES)))
    _CACHE["last_results"] = res
    out = np.empty((B, S, D), dtype=np.float32)
    for c in range(NCORES):
        b, g = c // GROUP, c % GROUP
        out[b][:, LCH * g:LCH * g + LCH] = res.results[c]["outT"].T.astype(
            np.float32)
    return out


# revision 4
# speedup vs baseline: 1.0009x; 1.0009x over previous
"""Causal self-attention on 8 Trainium2 NeuronCores.

Sharding (data + head parallel): core c handles batch b = c // 4 and the
4 heads [4g, 4g+4) where g = c % 4.  Each core projects q/k/v for its
heads (weights pre-sliced + pre-transposed on host), runs causal
attention, then the 4 cores of each batch AllGather the per-head
attention outputs (hd-major fp16) and each computes a disjoint
256-channel column slice of the output projection.

Pipeline (v3):
- x is DMA'd in 512-column slabs; projections run per-slab so the first
  attention chunk starts ~5us in instead of waiting for the full 4MB.
- One AllGather per 512-query chunk (8 small collectives spread across
  the kernel). Output projection consumes gathered chunks as soon as
  they land, scheduled >=1 full attention chunk after each AllGather
  fires so the in-order PE stream never waits on a collective; only the
  last chunk's gather is on the tail.
- fp16 data path, fp32 PSUM accumulation, f16 output (host casts).
- PSUM budget (8 banks): score tile 2 banks x 2 bufs; attention
  accumulators and projection matmuls alternate the "at"/"pp" slots.

Layouts per core:
  xT    (1024, 2048)  x[b].T                       (d on partitions)
  wqkT  (1024, 512)   [ (Wq[rows]/8).T | Wk[rows].T ]
  wvT   (1024, 256)   Wv[rows].T
  woT   (1024, 256)   Wo[rows].T with rows permuted to the AllGather
                      order: [pair p=0: rank r: heads 4r,4r+1] then
                      [pair p=1: rank r: heads 4r+2,4r+3]
  mask  (128, 128)    upper-triangular ones (k <= q)
  outT  (256, 2048)   out[b][:, cols].T  (f16)
"""

import os
import numpy as np

B, S, D, H = 2, 2048, 1024, 16
HD = D // H              # 64
NCORES = 8
GROUP = 4                # cores per batch
LHEADS = 4               # heads per core
LCH = LHEADS * HD        # 256 local channels
KT = D // 128            # 8 contraction tiles
ST = S // 128            # 16 sequence tiles
PAIRS = 2                # head pairs per core
CHUNK = 512              # q columns per attention pass
NCH = S // CHUNK         # 4

RECIP_FAST = os.environ.get("BASS_RECIP_FAST", "0") == "1"

_CACHE = {}


def _f16(a):
    return np.ascontiguousarray(a, dtype=np.float16)


def _build():
    import concourse.bacc as bacc
    import concourse.mybir as mybir
    import concourse.tile as tile

    f32 = mybir.dt.float32
    f16 = mybir.dt.float16
    Exp = mybir.ActivationFunctionType.Exp

    nc = bacc.Bacc(num_devices=NCORES)
    xT = nc.dram_tensor("xT", [D, S], f16, kind="ExternalInput")
    wqkT = nc.dram_tensor("wqkT", [D, 2 * LCH], f16, kind="ExternalInput")
    wvT = nc.dram_tensor("wvT", [D, LCH], f16, kind="ExternalInput")
    woT = nc.dram_tensor("woT", [D, LCH], f16, kind="ExternalInput")
    mask = nc.dram_tensor("mask", [128, 128], f16, kind="ExternalInput")
    outT = nc.dram_tensor("outT", [LCH, S], f16, kind="ExternalOutput")

    RG = [[0, 1, 2, 3], [4, 5, 6, 7]]

    with tile.TileContext(nc, num_cores=NCORES) as tc:
        with (
            tc.tile_pool(name="const", bufs=1) as const,
            tc.tile_pool(name="qkv", bufs=1) as qkv,
            tc.tile_pool(name="psum", bufs=1, space="PSUM") as psum,
            tc.tile_pool(name="dram", bufs=1, space="DRAM") as dram,
            tc.tile_pool(name="work", bufs=1) as work,
            tc.tile_pool(name="proj", bufs=1) as projp,
            tc.tile_pool(name="agp", bufs=1) as agp,
        ):
            mask_sb = const.tile([128, 128], f16)
            nc.sync.dma_start(mask_sb[:], mask[:])
            ones4 = const.tile([128, LHEADS], f32)
            nc.vector.memset(ones4[:], 1.0)

            cc_in = [[dram.tile([128, CHUNK], f16, name=f"ccin{p}{c}")
                      for c in range(NCH)] for p in range(PAIRS)]
            cc_out = [[dram.tile([GROUP * 128, CHUNK], f16, name=f"ccout{p}{c}")
                       for c in range(NCH)] for p in range(PAIRS)]

            qt = qkv.tile([128, PAIRS, S], f16)
            kt = qkv.tile([128, PAIRS, S], f16)
            v = qkv.tile([128, ST, LHEADS, 65], f16)

            # ---------------- input loads (slab-major for x) ----------------
            wqk, wv, xt = [], [], []
            for k in range(KT):
                tw = projp.tile([128, 2 * LCH], f16, name=f"wqk{k}")
                nc.sync.dma_start(tw[:], wqkT[128 * k:128 * k + 128, :])
                wqk.append(tw)
            for k in range(KT):
                tx = projp.tile([128, NCH, CHUNK], f16, name=f"xt{k}")
                xt.append(tx)
            for s in range(NCH):
                for k in range(KT):
                    nc.sync.dma_start(
                        xt[k][:, s, :],
                        xT[128 * k:128 * k + 128, CHUNK * s:CHUNK * s + CHUNK])
                if s == 0:
                    for k in range(KT):
                        tv = projp.tile([128, LCH], f16, name=f"wv{k}")
                        nc.sync.dma_start(tv[:], wvT[128 * k:128 * k + 128, :])
                        wv.append(tv)
            wo = projp.tile([128, KT, LCH], f16)
            nc.sync.dma_start(wo[:], woT[:].rearrange("(k p) n -> p k n", p=128))

            ptag = [0]

            def next_tag():
                t = "at" if ptag[0] % 2 == 0 else "pp"
                ptag[0] += 1
                return t

            def qk_proj(m, s):
                # m: 0,1 = q pair 0/1; 2,3 = k pair 0/1; s: 512-col slab
                dst = qt if m < 2 else kt
                pp = psum.tile([128, CHUNK], f32, tag=next_tag(),
                               name=f"qk{m}{s}")
                for k in range(KT):
                    nc.tensor.matmul(
                        pp[:], wqk[k][:, 128 * m:128 * m + 128], xt[k][:, s, :],
                        start=(k == 0), stop=(k == KT - 1))
                nc.vector.tensor_copy(
                    dst[:, m % 2, CHUNK * s:CHUNK * s + CHUNK], pp[:])

            def v_proj(j):
                vps = psum.tile([128, LCH], f32, tag=next_tag(), name=f"v{j}")
                for k in range(KT):
                    nc.tensor.matmul(
                        vps[:], xt[k][:, j // 4, 128 * (j % 4):128 * (j % 4) + 128],
                        wv[k][:], start=(k == 0), stop=(k == KT - 1))
                nc.vector.tensor_copy(
                    v[:, j, :, 64:65], ones4[:].rearrange("p (h o) -> p h o", o=1))
                nc.vector.tensor_copy(
                    v[:, j, :, 0:64], vps[:].rearrange("p (h e) -> p h e", h=LHEADS))

            ccin_last = [None]

            def stage_chunk(p, c, attps):
                """Normalize chunk c's accumulators, ship to the collective
                buffer, and AllGather this chunk."""
                asb = work.tile([65, 2 * CHUNK], f32, tag="asb", bufs=2,
                                name=f"asb{p}{c}")
                nc.vector.tensor_copy(asb[:], attps[:])
                rc = work.tile([65, 2 * CHUNK], f32, tag="rc", bufs=2,
                               name=f"rc{p}{c}")
                if RECIP_FAST:
                    nc.vector.reciprocal_approx_fast(rc[64:65, :], asb[64:65, :])
                else:
                    nc.vector.reciprocal(rc[64:65, :], asb[64:65, :])
                rc0 = work.tile([1, 2 * CHUNK], f32, tag="rc0", bufs=2,
                                name=f"rc0{p}{c}")
                nc.sync.dma_start(rc0[0:1, :], rc[64:65, :])
                bc = work.tile([64, 2 * CHUNK], f32, tag="bc", bufs=2,
                               name=f"bc{p}{c}")
                nc.gpsimd.partition_broadcast(bc[:], rc0[0:1, :])
                ao = work.tile([64, 2 * CHUNK], f16, tag="ao", bufs=2,
                               name=f"ao{p}{c}")
                nc.vector.tensor_mul(ao[:, :], asb[0:64, :], bc[:, :])
                for h in range(2):
                    ccin_last[0] = nc.sync.dma_start(
                        cc_in[p][c][64 * h:64 * h + 64, :],
                        ao[:, CHUNK * h:CHUNK * h + CHUNK])
                nc.gpsimd.collective_compute(
                    "AllGather", mybir.AluOpType.bypass, replica_groups=RG,
                    ins=[cc_in[p][c][:]], outs=[cc_out[p][c][:]])

            def attn_chunk(p, c):
                q0 = CHUNK * c
                nj = 4 * c + 4
                attps = psum.tile([65, 2 * CHUNK], f32, tag=next_tag(),
                                  name=f"att{p}{c}")
                for j in range(nj):
                    qs = max(q0, 128 * j)
                    n = q0 + CHUNK - qs
                    off = qs - q0
                    sc = psum.tile([128, 1024], f32, tag="sc", bufs=2,
                                   name=f"sc{p}{c}{j}")
                    for h in range(2):
                        pb = 64 * h
                        nc.tensor.matmul(
                            sc[:, 512 * h:512 * h + n],
                            kt[pb:pb + 64, p, 128 * j:128 * j + 128],
                            qt[pb:pb + 64, p, qs:qs + n],
                            start=True, stop=True)
                    ex = work.tile([128, 1024], f16, tag="ex", bufs=3,
                                   name=f"ex{p}{c}{j}")
                    nc.scalar.activation(
                        ex[:].rearrange("q (t x) -> q t x", t=2)[:, :, 0:n],
                        sc[:].rearrange("q (t x) -> q t x", t=2)[:, :, 0:n],
                        Exp)
                    if qs == 128 * j:  # diagonal tile: causal mask
                        for h in range(2):
                            nc.vector.tensor_mul(
                                ex[:, 512 * h:512 * h + 128],
                                ex[:, 512 * h:512 * h + 128], mask_sb[:])
                    for h in range(2):
                        nc.tensor.matmul(
                            attps[:, 512 * h + off:512 * h + CHUNK],
                            v[:, j, 2 * p + h, :],
                            ex[:, 512 * h:512 * h + n],
                            start=(j == 0), stop=(j == nj - 1))
                stage_chunk(p, c, attps)

            oacc = [agp.tile([128, 2 * CHUNK], f32, name=f"oacc{ct}")
                    for ct in range(2)]
            agt = {}

            def fetch_ag(ph, c):
                """Prefetch the gathered chunk (ph, c) into SBUF."""
                ts = []
                for r in range(GROUP):
                    t = agp.tile([128, CHUNK], f16, tag=f"ag{ph}{c % 2}{r}",
                                 bufs=1, name=f"ag{ph}{c}{r}")
                    dma = nc.sync.dma_start(
                        t[:], cc_out[ph][c][128 * r:128 * r + 128, :])
                    if ccin_last[0] is not None:
                        tile.add_dep_helper(
                            dma.ins, ccin_last[0].ins, sync=True,
                            reason="gather prefetch after prior staging")
                    ts.append(t)
                agt[(ph, c)] = ts

            def out_mms(pp, ph, c, ct, first, last):
                cc = c % 2
                ags = agt[(ph, c)]
                for k4 in range(GROUP):
                    nc.tensor.matmul(
                        pp[:, 512 * cc:512 * cc + 512],
                        wo[:, 4 * ph + k4, 128 * ct:128 * ct + 128],
                        ags[k4][:],
                        start=(first and k4 == 0), stop=(last and k4 == GROUP - 1))

            def out_full(hb):
                """Output projection for block hb, both pairs gathered."""
                for ct in range(2):
                    pp = psum.tile([128, 2 * CHUNK], f32, tag=next_tag(),
                                   name=f"opf{hb}{ct}")
                    for cc in range(2):
                        for ph in range(PAIRS):
                            out_mms(pp, ph, 2 * hb + cc, ct,
                                    first=(ph == 0), last=(ph == PAIRS - 1))
                    ot = agp.tile([128, 2 * CHUNK], f16, tag=f"ot{ct}", bufs=2,
                                  name=f"otf{hb}{ct}")
                    nc.vector.tensor_copy(ot[:], pp[:])
                    nc.sync.dma_start(
                        outT[128 * ct:128 * ct + 128,
                             2 * CHUNK * hb:2 * CHUNK * hb + 2 * CHUNK], ot[:])

            def out_part(hb, ph, first):
                """Half of block hb's output projection (pair ph only).
                first -> park in SBUF f32; else add + write out."""
                pps = []
                for ct in range(2):
                    pp = psum.tile([128, 2 * CHUNK], f32, tag=next_tag(),
                                   name=f"opp{hb}{ph}{ct}")
                    pps.append(pp)
                for cc in range(2):
                    for ct in range(2):
                        out_mms(pps[ct], ph, 2 * hb + cc, ct, True, True)
                for ct in range(2):
                    if first:
                        nc.vector.tensor_copy(oacc[ct][:], pps[ct][:])
                    else:
                        ot = agp.tile([128, 2 * CHUNK], f16, tag=f"ot{ct}",
                                      bufs=2, name=f"otp{hb}{ct}")
                        nc.vector.tensor_add(ot[:], pps[ct][:], oacc[ct][:])
                        nc.sync.dma_start(
                            outT[128 * ct:128 * ct + 128,
                                 2 * CHUNK * hb:2 * CHUNK * hb + 2 * CHUNK],
                            ot[:])

            # ---------------- schedule ----------------
            def proj_slab(s):
                qk_proj(0, s)         # pair-0 q
                qk_proj(2, s)         # pair-0 k
                for j in range(4 * s, 4 * s + 4):
                    v_proj(j)
                qk_proj(1, s)         # pair-1 q
                qk_proj(3, s)         # pair-1 k

            proj_slab(0)
            attn_chunk(0, 0)          # -> AG(0,0)
            proj_slab(1)
            attn_chunk(1, 0)          # -> AG(1,0)
            proj_slab(2)
            attn_chunk(0, 1)          # -> AG(0,1)
            proj_slab(3)
            attn_chunk(1, 1)          # -> AG(1,1)
            attn_chunk(1, 2)          # -> AG(1,2)
            for pc in ((0, 0), (0, 1), (1, 0), (1, 1)):
                fetch_ag(*pc)
            attn_chunk(1, 3)          # -> AG(1,3)
            fetch_ag(1, 2)
            out_full(0)
            attn_chunk(0, 2)          # -> AG(0,2)
            fetch_ag(1, 3)
            out_part(1, 1, True)
            attn_chunk(0, 3)          # -> AG(0,3)
            fetch_ag(0, 2)
            fetch_ag(0, 3)
            out_part(1, 0, False)

    nc.compile()
    return nc


def _gather_perm():
    """d-channel permutation matching the AllGather layout."""
    perm = []
    for p in range(PAIRS):
        for r in range(GROUP):
            for h in range(2):
                head = 4 * r + 2 * p + h
                perm.extend(range(HD * head, HD * head + HD))
    return np.array(perm)


def _shard_inputs(x, Wq, Wk, Wv, Wo):
    x = np.asarray(x, dtype=np.float32)
    Wq = np.asarray(Wq, dtype=np.float32)
    Wk = np.asarray(Wk, dtype=np.float32)
    Wv = np.asarray(Wv, dtype=np.float32)
    Wo = np.asarray(Wo, dtype=np.float32)
    mask = np.triu(np.ones((128, 128), dtype=np.float16))
    perm = _gather_perm()
    in_maps = []
    for c in range(NCORES):
        b, g = c // GROUP, c % GROUP
        rows = slice(LCH * g, LCH * g + LCH)
        in_maps.append({
            "xT": _f16(x[b].T),
            "wqkT": _f16(np.concatenate([Wq[rows] / 8.0, Wk[rows]], axis=0).T),
            "wvT": _f16(Wv[rows].T),
            "woT": _f16(Wo[rows].T[perm, :]),
            "mask": mask,
        })
    return in_maps


def kernel(x, Wq, Wk, Wv, Wo):
    from concourse.bass_utils import run_bass_kernel_spmd

    if "nc" not in _CACHE:
        _CACHE["nc"] = _build()
    nc = _CACHE["nc"]
    in_maps = _shard_inputs(x, Wq, Wk, Wv, Wo)
    res = run_bass_kernel_spmd(nc, in_maps, core_ids=list(range(NCORES)))
    _CACHE["last_results"] = res
    out = np.empty((B, S, D), dtype=np.float32)
    for c in range(NCORES):
        b, g = c // GROUP, c % GROUP
        out[b][:, LCH * g:LCH * g + LCH] = res.results[c]["outT"].T.astype(
            np.float32)
    return out
